# revision 5
# baseline (speedup 1.0000x reference)
"""Trainium2 Bass kernel for nn_DCTLayer: 8x8 block DCT-II followed by its exact
inverse (torch_dct norm=None convention). The DCT->IDCT round trip is the
identity map in exact arithmetic, so the layer reduces to the block-layout
permutation (B, C, H, W) -> (B, C, 1, H, W) where out[b, c, 0] is the row-major
flatten of the (H/8, W/8, 8, 8) block view of the input. Computing the
permutation exactly is strictly more accurate than the reference's own fp32 FFT
round trip (rel err ~1e-7 against it).

Distribution (pure data parallelism over batch, 8 cores, no communication):
  - core k handles batches 4k..4k+4 = 12 images of 512x512 f32 (12 MiB).
  - Input viewed as [768, 4096]: each row chunk = 8 consecutive image rows
    (16 KiB, DRAM-contiguous) -> one SBUF partition.
  - On-chip shuffle per partition (vector engine, 4D access patterns):
    free-dim permutation (r, bw, c) -> (bw, r, c) with r=8 image rows,
    bw=64 block-columns, c=8.
  - Output [768, 4096] is DRAM-contiguous per partition too, so both DMAs run
    at full descriptor efficiency. The binding resource is the per-engine SDMA
    port rate (~26.5 GB/s x 16 engines): 1.5 MiB/engine ~= 59 us.

Raw bass (no TileContext): the Tile end-of-context barrier costs a fixed
~8 us of EVENT_SEMAPHORE drain on every sequencer after the last DMA; with
manual semaphores the kernel ends ~0.3 us after the last store lands.
Loads are split 3/3 over the two HWDGE rings (SP=sync, ACT=scalar) so the
DMA engines fill from both descriptor generators at once; all loads are
issued up front (the full 96 KiB/partition input stays resident in SBUF),
stores are split per half-tile over both rings behind the vector shuffle.
"""

import numpy as np

_B, _C, _H, _W = 32, 3, 512, 512
_N_CORES = 8
_ROWS = (_B // _N_CORES) * _C * (_H // 8)  # 768 row chunks per core
_COLS = 8 * _W                             # 4096 f32 per chunk
_N_TILES = _ROWS // 128                    # 6 tiles of [128, 4096]
_OUT_BUFS = 4

_nc_cache = None


def _build():
    import concourse.mybir as mybir
    from concourse import bacc

    nc = bacc.Bacc(
        "TRN2", target_bir_lowering=False, debug=False, num_devices=_N_CORES
    )
    x = nc.dram_tensor(
        "x", (_ROWS, _COLS), mybir.dt.float32, kind="ExternalInput"
    ).ap()
    y = nc.dram_tensor(
        "y", (_ROWS, _COLS), mybir.dt.float32, kind="ExternalOutput"
    ).ap()

    f32 = mybir.dt.float32
    with (
        nc.sbuf_tensor([128, _N_TILES * _COLS], f32) as tin,
        nc.sbuf_tensor([128, _OUT_BUFS * _COLS], f32) as tout,
        nc.semaphore() as sem_ld_sp,   # loads on sync (SP ring)
        nc.semaphore() as sem_ld_act,  # loads on scalar (ACT ring)
        nc.semaphore() as sem_cp,      # vector copies
        nc.semaphore() as sem_st_sp,   # stores on sync
        nc.semaphore() as sem_st_act,  # stores on scalar
    ):
        # All 6 loads issued up front, alternating rings: L0,L2,L4 -> sync,
        # L1,L3,L5 -> scalar. tin column block t holds DRAM rows [128t,128t+128).
        for t in range(_N_TILES):
            eng = nc.sync if t % 2 == 0 else nc.scalar
            sem = sem_ld_sp if t % 2 == 0 else sem_ld_act
            eng.dma_start(
                out=tin[:, t * _COLS:(t + 1) * _COLS],
                in_=x[t * 128:(t + 1) * 128, :],
                single_packet=True,
            ).then_inc(sem, 16)

        # Vector: per tile, 2 half-shuffles (bw split). Copy of tile t writes
        # out-buffer t % _OUT_BUFS; for t >= _OUT_BUFS first wait until that
        # buffer's two stores (tile t - _OUT_BUFS) have completed.
        for t in range(_N_TILES):
            sem = sem_ld_sp if t % 2 == 0 else sem_ld_act
            nc.vector.wait_ge(sem, 16 * (t // 2 + 1))
            if t >= _OUT_BUFS:
                done = t - _OUT_BUFS + 1
                nc.vector.wait_ge(sem_st_sp, 16 * done)
                nc.vector.wait_ge(sem_st_act, 16 * done)
            b = t % _OUT_BUFS
            src = tin[:, t * _COLS:(t + 1) * _COLS].rearrange(
                "p (r bw c) -> p bw r c", r=8, bw=64, c=8
            )
            dst = tout[:, b * _COLS:(b + 1) * _COLS].rearrange(
                "p (bw r c) -> p bw r c", bw=64, r=8, c=8
            )
            for s in range(2):
                bws = slice(s * 32, (s + 1) * 32)
                nc.vector.tensor_copy(out=dst[:, bws], in_=src[:, bws]).then_inc(
                    sem_cp, 1
                )

        # Stores: half-tile (t, s) -> scalar for s=0, sync for s=1.
        # Store (t, s) needs copy count 2t + s + 1.
        for t in range(_N_TILES):
            b = t % _OUT_BUFS
            for s, (eng, sem) in enumerate(
                ((nc.scalar, sem_st_act), (nc.sync, sem_st_sp))
            ):
                eng.wait_ge(sem_cp, 2 * t + s + 1)
                eng.dma_start(
                    out=y[t * 128:(t + 1) * 128, s * 2048:(s + 1) * 2048],
                    in_=tout[:, b * _COLS + s * 2048:b * _COLS + (s + 1) * 2048],
                    single_packet=True,
                ).then_inc(sem, 16)

        # Epilogue: gpsimd waits for all stores to land, then resets DMA
        # queue state and clears the semaphores so the NEFF can be
        # re-executed (sems are not cleared on allocation or at load).
        nc.gpsimd.wait_ge(sem_st_sp, 16 * _N_TILES)
        nc.gpsimd.wait_ge(sem_st_act, 16 * _N_TILES)
        nc.gpsimd.dma_reset()
        for sem in (sem_ld_sp, sem_ld_act, sem_cp, sem_st_sp, sem_st_act):
            nc.gpsimd.sem_clear(sem)

        nc.compile()
    return nc


def kernel(x: np.ndarray) -> np.ndarray:
    from concourse import bass_utils

    global _nc_cache
    if _nc_cache is None:
        _nc_cache = _build()
    nc = _nc_cache

    x = np.ascontiguousarray(x, dtype=np.float32)
    assert x.shape == (_B, _C, _H, _W), x.shape
    xs = x.reshape(_N_CORES, _ROWS, _COLS)
    in_maps = [{"x": xs[k]} for k in range(_N_CORES)]
    res = bass_utils.run_bass_kernel_spmd(
        nc, in_maps, core_ids=list(range(_N_CORES))
    )
    ys = np.stack([res.results[k]["y"] for k in range(_N_CORES)], axis=0)
    return ys.reshape(_B, _C, 1, _H, _W)
